# revision 2
# baseline (speedup 1.0000x reference)
"""Trainium2 Bass kernel for nn_LocalGeometryLoss (v3).

Same math as v2 (host-normalized inputs, sharded row-blocks, device top-k +
gather + masked affinity*d2 reduction), with the v2 trace's bottlenecks fixed:

  - The [512, 8192] Gram block is computed with fp8e4m3 DoubleRow matmuls
    (inputs scaled by 16; sim scale 256 only permutes nothing: top-k is
    scale-invariant). Halves both PE time and the hpT load bytes.
    Validated numerically: rel err vs fp32 reference 1.6-5.8e-3.
  - Sim rows stored fp16 (fp32 PSUM -> fp16 SBUF on the ACT engine, two
    512-chunks per copy to amortize the ~185ns ACT fixed cost).
  - Top-k on DVE: MAX8 + FIND_INDEX8 over the fp16 row (uint16 indices).
    fp16, not bf16: bf16 rounding creates top-k ties whose low-index-first
    resolution systematically biases the valid-neighbor count (~1.1e-2
    rel err); fp16 keeps selection error at ~2e-3.
  - Neighbor-dot products on GpSimd (bf16), their row-sums on the ACT
    accumulator, label-compare reads the gathered label column directly.
  - m-tile 0's matmuls ride the 16-chunk hpT load; m1..m3 follow densely,
    so the first top-k chain starts right after the load drains.
"""

import numpy as np
import ml_dtypes

import concourse.bass as bass
import concourse.bacc as bacc
import concourse.mybir as mybir
from concourse import tile
from concourse.bass_utils import run_bass_kernel_spmd

FP = mybir.dt.float32
BF = mybir.dt.bfloat16
F16 = mybir.dt.float16
F8 = mybir.dt.float8e4
U16 = mybir.dt.uint16
U32 = mybir.dt.uint32

B_PREV = 8192
B_CURR = 4096
D = 768
KNBR = 5
WEIGHT = 0.5
N_CORES = 8
ROWS_PER_CORE = B_CURR // N_CORES          # 512
M_TILES = ROWS_PER_CORE // 128             # 4
K3 = 3                                     # 3 fp8 DoubleRow chunks of 256
NC_CHUNK = 512
N_CHUNKS = B_PREV // NC_CHUNK              # 16
TBL_W = 772                                # 768 hc + 1 label + 3 pad
ACT = mybir.ActivationFunctionType
ALU = mybir.AluOpType
PM = mybir.MatmulPerfMode
BF_NP = ml_dtypes.bfloat16
F8_NP = ml_dtypes.float8_e4m3
F8_SCALE = 16.0

_CACHE = {}


def _build():
    nc = bacc.Bacc("TRN2", target_bir_lowering=False, debug=False,
                   num_devices=N_CORES, num_swdge_queues=4)

    # fp8 hp, chunk-major: [n, p, k3, i, j2] = hp8[512n+j2, 256k3+128i+p]
    hpT_d = nc.dram_tensor("hpT", [N_CHUNKS, 128, K3 * 2 * NC_CHUNK], F8,
                           kind="ExternalInput").ap()
    # own 512 rows: [p, k3, i, i2] = hp8[r0+i2, 256k3+128i+p]
    lhsT_d = nc.dram_tensor("lhsT", [128, K3 * 2 * ROWS_PER_CORE], F8,
                            kind="ExternalInput").ap()
    # own hc rows, normalized bf16: [p, m, d] = hcn[r0+128m+p, d]
    s_d = nc.dram_tensor("s", [128, M_TILES * D], BF, kind="ExternalInput").ap()
    # [4096, 772] bf16: cols 0:768 normalized hc row j, col 768 labels_prev[j]
    tbl = nc.dram_tensor("tbl", [B_CURR, TBL_W], BF, kind="ExternalInput").ap()
    # labels of own 512 prev rows, [128, 4]
    lbl_d = nc.dram_tensor("lbl", [128, M_TILES], FP, kind="ExternalInput").ap()

    partial = nc.dram_tensor("partial", [1, 1], FP, kind="ExternalOutput").ap()

    with tile.TileContext(nc) as tc:
        sb = tc.alloc_tile_pool(name="sb", bufs=1)
        simp = tc.alloc_tile_pool(name="simp", bufs=3)
        small = tc.alloc_tile_pool(name="small", bufs=3)
        gathp = tc.alloc_tile_pool(name="gathp", bufs=4)
        psp = tc.alloc_tile_pool(name="psp", bufs=3, space="PSUM")
        psp1 = tc.alloc_tile_pool(name="psp1", bufs=1, space="PSUM")

        hpT = sb.tile([128, N_CHUNKS, K3, 2, NC_CHUNK], F8)
        lhsT = sb.tile([128, K3, 2, ROWS_PER_CORE], F8)
        s_bf = sb.tile([128, M_TILES, D], BF)
        lbl_sb = sb.tile([128, M_TILES], FP)
        acc = sb.tile([128, M_TILES * KNBR], FP)
        ones = sb.tile([128, 1], FP)
        twos = sb.tile([128, 1], FP)

        nc.vector.memset(ones[:], 1.0)
        nc.vector.memset(twos[:], 2.0)
        nc.sync.dma_start(lbl_sb[:], lbl_d[:])
        nc.sync.dma_start(lhsT[:], lhsT_d.rearrange(
            "p (k i r) -> p k i r", k=K3, i=2))
        nc.sync.dma_start(s_bf[:], s_d.rearrange("p (m d) -> p m d", m=M_TILES))

        sims = [None] * M_TILES

        def mm_pair(m, pair):
            ps = psp.tile([128, 2 * NC_CHUNK], FP, tag="ps")
            for c in (0, 1):
                n = 2 * pair + c
                for k in range(K3):
                    nc.tensor.matmul(
                        ps[:, NC_CHUNK * c:NC_CHUNK * (c + 1)],
                        lhsT[:, k, :, 128 * m:128 * (m + 1)],
                        hpT[:, n, k, :, :],
                        start=(k == 0), stop=(k == K3 - 1),
                        perf_mode=PM.DoubleRow)
            nc.scalar.copy(
                sims[m][:, 2 * NC_CHUNK * pair:2 * NC_CHUNK * (pair + 1)],
                ps[:])

        def topk_gather(m):
            v8 = small.tile([128, 8], F16, tag="v8")
            nc.vector.max(out=v8[:], in_=sims[m][:])
            i8 = small.tile([128, 8], U16, tag="i8")
            nc.vector.max_index(out=i8[:], in_max=v8[:], in_values=sims[m][:])
            i32 = small.tile([128, 8], U32, tag="i32")
            nc.vector.tensor_copy(i32[:], i8[:])

            # neighbor slots 1..5; clamp indices to <4096 for the gather
            jc = small.tile([128, KNBR], U32, tag="jc")
            nc.vector.tensor_scalar(out=jc[:], in0=i32[:, 1:6], scalar1=B_CURR - 1,
                                    scalar2=None, op0=ALU.min)
            msk = small.tile([128, KNBR], FP, tag="msk")
            nc.vector.tensor_scalar(out=msk[:], in0=i32[:, 1:6], scalar1=B_CURR,
                                    scalar2=None, op0=ALU.is_lt)

            dots = small.tile([128, KNBR], FP, tag="dots")
            eqv = small.tile([128, KNBR], FP, tag="eqv")
            for s in range(KNBR):
                g = gathp.tile([128, TBL_W], BF, tag="g")
                nc.gpsimd.indirect_dma_start(
                    out=g[:], out_offset=None, in_=tbl[:],
                    in_offset=bass.IndirectOffsetOnAxis(ap=jc[:, s:s + 1], axis=0))
                nc.vector.tensor_scalar(out=eqv[:, s:s + 1], in0=g[:, D:D + 1],
                                        scalar1=lbl_sb[:, m:m + 1],
                                        scalar2=None, op0=ALU.is_equal)
                prod = gathp.tile([128, D], BF, tag="prod")
                nc.gpsimd.tensor_tensor(out=prod[:], in0=g[:, :D],
                                        in1=s_bf[:, m, :], op=ALU.mult)
                actout = gathp.tile([128, D], BF, tag="actout")
                nc.scalar.activation(actout[:], prod[:], ACT.Copy,
                                     accum_out=dots[:, s:s + 1])

            # d2 = relu(2 - 2 cos); both sides pre-normalized so dot == cos
            d2 = small.tile([128, KNBR], FP, tag="d2")
            nc.scalar.activation(d2[:], dots[:], ACT.Relu, bias=twos[:, :1],
                                 scale=-2.0)

            # e = 2*eqv - 1, masked
            e5 = small.tile([128, KNBR], FP, tag="e5")
            nc.vector.tensor_scalar(out=e5[:], in0=eqv[:], scalar1=2.0,
                                    scalar2=-1.0, op0=ALU.mult, op1=ALU.add)
            em = small.tile([128, KNBR], FP, tag="em")
            nc.vector.tensor_tensor(out=em[:], in0=e5[:], in1=msk[:], op=ALU.mult)
            nc.vector.tensor_tensor(out=acc[:, KNBR * m:KNBR * (m + 1)],
                                    in0=em[:], in1=d2[:], op=ALU.mult)

        # ---- hpT load with m0 matmuls riding it ----
        sims[0] = simp.tile([128, B_PREV], F16, tag="sim", name="sim0")
        for pair in range(N_CHUNKS // 2):
            nc.sync.dma_start(hpT[:, 2 * pair], hpT_d[2 * pair].rearrange(
                "p (k i j) -> p k i j", k=K3, i=2))
            nc.sync.dma_start(hpT[:, 2 * pair + 1], hpT_d[2 * pair + 1].rearrange(
                "p (k i j) -> p k i j", k=K3, i=2))
            mm_pair(0, pair)

        topk_gather(0)
        for m in (1, 2, 3):
            sims[m] = simp.tile([128, B_PREV], F16, tag="sim", name=f"sim{m}")
            for pair in range(N_CHUNKS // 2):
                mm_pair(m, pair)
            topk_gather(m)

        # ---- final reduction: acc [128, 20] -> scalar ----
        rowsum = small.tile([128, 1], FP, tag="rowsum")
        nc.vector.tensor_reduce(out=rowsum[:], in_=acc[:],
                                axis=mybir.AxisListType.X, op=ALU.add)
        pps = psp1.tile([1, 1], FP, tag="pps")
        nc.tensor.matmul(pps[:], ones[:], rowsum[:], start=True, stop=True)
        res = small.tile([1, 1], FP, tag="res")
        nc.scalar.copy(res[:], pps[:])
        sc = small.tile([1, 1], FP, tag="sc")
        nc.vector.tensor_scalar_mul(sc[:], res[:], WEIGHT / (B_CURR * B_CURR))
        nc.sync.dma_start(partial[:], sc[:])

        for p in (psp1, psp, gathp, small, simp, sb):
            p.release()

    nc.compile()
    return nc


def _get_nc():
    if "nc" not in _CACHE:
        _CACHE["nc"] = _build()
    return _CACHE["nc"]


def _in_maps(hidden_current, hidden_previous, labels_previous):
    hp = np.asarray(hidden_previous, dtype=np.float32)
    hc = np.asarray(hidden_current, dtype=np.float32)
    lp = np.asarray(labels_previous).astype(np.float32)

    hpn = hp / np.maximum(np.linalg.norm(hp, axis=1, keepdims=True), 1e-12)
    hcn = hc / np.maximum(np.linalg.norm(hc, axis=1, keepdims=True), 1e-12)
    hp8 = (hpn * F8_SCALE).astype(F8_NP)
    hcn_bf = hcn.astype(BF_NP)

    # [n, p, k3, i, j2] = hp8[512n+j2, 256k3+128i+p]
    hpT_host = np.ascontiguousarray(
        hp8.reshape(N_CHUNKS, NC_CHUNK, K3, 2, 128).transpose(0, 4, 2, 3, 1)
    ).reshape(N_CHUNKS, 128, K3 * 2 * NC_CHUNK)

    tbl = np.zeros((B_CURR, TBL_W), dtype=BF_NP)
    tbl[:, :D] = hcn_bf
    tbl[:, D] = lp[:B_CURR].astype(BF_NP)

    in_maps = []
    for c in range(N_CORES):
        r0 = c * ROWS_PER_CORE
        lhsT_host = np.ascontiguousarray(
            hp8[r0:r0 + ROWS_PER_CORE].reshape(ROWS_PER_CORE, K3, 2, 128)
            .transpose(3, 1, 2, 0)
        ).reshape(128, K3 * 2 * ROWS_PER_CORE)
        s_host = np.ascontiguousarray(
            hcn_bf[r0:r0 + ROWS_PER_CORE].reshape(M_TILES, 128, D)
            .transpose(1, 0, 2)
        ).reshape(128, M_TILES * D)
        lbl_host = np.ascontiguousarray(
            lp[r0:r0 + ROWS_PER_CORE].reshape(M_TILES, 128).T)
        in_maps.append({
            "hpT": hpT_host,
            "lhsT": lhsT_host,
            "s": s_host,
            "tbl": tbl,
            "lbl": lbl_host,
        })
    return in_maps


def _run(hidden_current, hidden_previous, labels_previous, trace=False):
    nc = _get_nc()
    in_maps = _in_maps(hidden_current, hidden_previous, labels_previous)
    out = run_bass_kernel_spmd(nc, in_maps, list(range(N_CORES)), trace=trace)
    total = np.float32(0.0)
    for c in range(N_CORES):
        total += out.results[c]["partial"][0, 0]
    return np.asarray(total, dtype=np.float32), out


def kernel(hidden_current, hidden_previous, labels_current, labels_previous):
    result, _ = _run(hidden_current, hidden_previous, labels_previous)
    return result


# revision 3
# speedup vs baseline: 1.1723x; 1.1723x over previous
"""Trainium2 Bass kernel for nn_LocalGeometryLoss (v3).

Same math as v2 (host-normalized inputs, sharded row-blocks, device top-k +
gather + masked affinity*d2 reduction), with the v2 trace's bottlenecks fixed:

  - The [512, 8192] Gram block is computed with fp8e4m3 DoubleRow matmuls
    (inputs scaled by 16; sim scale 256 only permutes nothing: top-k is
    scale-invariant). Halves both PE time and the hpT load bytes.
    Validated numerically: rel err vs fp32 reference 1.6-5.8e-3.
  - Sim rows stored fp16 (fp32 PSUM -> fp16 SBUF on the ACT engine, two
    512-chunks per copy to amortize the ~185ns ACT fixed cost).
  - Top-k on DVE: MAX8 + FIND_INDEX8 over the fp16 row (uint16 indices).
    fp16, not bf16: bf16 rounding creates top-k ties whose low-index-first
    resolution systematically biases the valid-neighbor count (~1.1e-2
    rel err); fp16 keeps selection error at ~2e-3.
  - Neighbor-dot products on GpSimd (bf16), their row-sums on the ACT
    accumulator, label-compare reads the gathered label column directly.
  - m-tile 0's matmuls ride the 16-chunk hpT load; m1..m3 follow densely,
    so the first top-k chain starts right after the load drains.
"""

import numpy as np
import ml_dtypes

import concourse.bass as bass
import concourse.bacc as bacc
import concourse.mybir as mybir
from concourse import tile
from concourse.bass_utils import run_bass_kernel_spmd

FP = mybir.dt.float32
BF = mybir.dt.bfloat16
F16 = mybir.dt.float16
F8 = mybir.dt.float8e4
U16 = mybir.dt.uint16
U32 = mybir.dt.uint32

B_PREV = 8192
B_CURR = 4096
D = 768
KNBR = 5
WEIGHT = 0.5
N_CORES = 8
ROWS_PER_CORE = B_CURR // N_CORES          # 512
M_TILES = ROWS_PER_CORE // 128             # 4
K3 = 3                                     # 3 fp8 DoubleRow chunks of 256
NC_CHUNK = 512
N_CHUNKS = B_PREV // NC_CHUNK              # 16
TBL_W = 772                                # 768 hc + 1 label + 3 pad
ACT = mybir.ActivationFunctionType
ALU = mybir.AluOpType
PM = mybir.MatmulPerfMode
BF_NP = ml_dtypes.bfloat16
F8_NP = ml_dtypes.float8_e4m3
F8_SCALE = 16.0

_CACHE = {}


def _build():
    nc = bacc.Bacc("TRN2", target_bir_lowering=False, debug=False,
                   num_devices=N_CORES, num_swdge_queues=4)

    # fp8 hp, chunk-major: [n, p, k3, i, j2] = hp8[512n+j2, 256k3+128i+p]
    hpT_d = nc.dram_tensor("hpT", [N_CHUNKS, 128, K3 * 2 * NC_CHUNK], F8,
                           kind="ExternalInput").ap()
    # own 512 rows: [p, k3, i, i2] = hp8[r0+i2, 256k3+128i+p]
    lhsT_d = nc.dram_tensor("lhsT", [128, K3 * 2 * ROWS_PER_CORE], F8,
                            kind="ExternalInput").ap()
    # own hc rows, normalized bf16: [p, m, d] = hcn[r0+128m+p, d]
    s_d = nc.dram_tensor("s", [128, M_TILES * D], BF, kind="ExternalInput").ap()
    # [4096, 772] bf16: cols 0:768 normalized hc row j, col 768 labels_prev[j]
    tbl = nc.dram_tensor("tbl", [B_CURR, TBL_W], BF, kind="ExternalInput").ap()
    # labels of own 512 prev rows, [128, 4]
    lbl_d = nc.dram_tensor("lbl", [128, M_TILES], FP, kind="ExternalInput").ap()

    partial = nc.dram_tensor("partial", [1, 1], FP, kind="ExternalOutput").ap()

    with tile.TileContext(nc) as tc:
        sb = tc.alloc_tile_pool(name="sb", bufs=1)
        simp = tc.alloc_tile_pool(name="simp", bufs=4)
        small = tc.alloc_tile_pool(name="small", bufs=3)
        gathp = tc.alloc_tile_pool(name="gathp", bufs=4)
        psp = tc.alloc_tile_pool(name="psp", bufs=3, space="PSUM")
        psp1 = tc.alloc_tile_pool(name="psp1", bufs=1, space="PSUM")

        hpT = sb.tile([128, N_CHUNKS, K3, 2, NC_CHUNK], F8)
        lhsT = sb.tile([128, K3, 2, ROWS_PER_CORE], F8)
        s_bf = sb.tile([128, M_TILES, D], BF)
        lbl_sb = sb.tile([128, M_TILES], FP)
        acc = sb.tile([128, M_TILES * KNBR], FP)
        ones = sb.tile([128, 1], FP)
        twos = sb.tile([128, 1], FP)

        nc.vector.memset(ones[:], 1.0)
        nc.vector.memset(twos[:], 2.0)
        nc.sync.dma_start(lhsT[:], lhsT_d.rearrange(
            "p (k i r) -> p k i r", k=K3, i=2))
        nc.sync.dma_start(s_bf[:], s_d.rearrange("p (m d) -> p m d", m=M_TILES))
        nc.sync.dma_start(lbl_sb[:], lbl_d[:])

        sims = [None] * M_TILES

        def mm_pair(m, pair):
            ps = psp.tile([128, 2 * NC_CHUNK], FP, tag="ps")
            for c in (0, 1):
                n = 2 * pair + c
                for k in range(K3):
                    nc.tensor.matmul(
                        ps[:, NC_CHUNK * c:NC_CHUNK * (c + 1)],
                        lhsT[:, k, :, 128 * m:128 * (m + 1)],
                        hpT[:, n, k, :, :],
                        start=(k == 0), stop=(k == K3 - 1),
                        perf_mode=PM.DoubleRow)
            nc.scalar.copy(
                sims[m][:, 2 * NC_CHUNK * pair:2 * NC_CHUNK * (pair + 1)],
                ps[:])

        def topk_gather(m):
            v8 = small.tile([128, 8], F16, tag="v8")
            nc.vector.max(out=v8[:], in_=sims[m][:])
            i8 = small.tile([128, 8], U16, tag="i8")
            nc.vector.max_index(out=i8[:], in_max=v8[:], in_values=sims[m][:])
            i32 = small.tile([128, 8], U32, tag="i32")
            nc.vector.tensor_copy(i32[:], i8[:])

            # neighbor slots 1..5; clamp indices to <4096 for the gather
            jc = small.tile([128, KNBR], U32, tag="jc")
            nc.vector.tensor_scalar(out=jc[:], in0=i32[:, 1:6], scalar1=B_CURR - 1,
                                    scalar2=None, op0=ALU.min)
            msk = small.tile([128, KNBR], FP, tag="msk")
            nc.vector.tensor_scalar(out=msk[:], in0=i32[:, 1:6], scalar1=B_CURR,
                                    scalar2=None, op0=ALU.is_lt)

            dots = small.tile([128, KNBR], FP, tag="dots")
            lblg = small.tile([128, KNBR], FP, tag="lblg")
            for s in range(KNBR):
                g = gathp.tile([128, TBL_W], BF, tag="g")
                nc.gpsimd.indirect_dma_start(
                    out=g[:], out_offset=None, in_=tbl[:],
                    in_offset=bass.IndirectOffsetOnAxis(ap=jc[:, s:s + 1], axis=0))
                nc.vector.tensor_copy(lblg[:, s:s + 1], g[:, D:D + 1])
                prod = gathp.tile([128, D], BF, tag="prod")
                nc.gpsimd.tensor_tensor(out=prod[:], in0=g[:, :D],
                                        in1=s_bf[:, m, :], op=ALU.mult)
                actout = gathp.tile([128, D], BF, tag="actout")
                nc.scalar.activation(actout[:], prod[:], ACT.Copy,
                                     accum_out=dots[:, s:s + 1])
            eqv = small.tile([128, KNBR], FP, tag="eqv")
            nc.vector.tensor_scalar(out=eqv[:], in0=lblg[:],
                                    scalar1=lbl_sb[:, m:m + 1],
                                    scalar2=None, op0=ALU.is_equal)

            # d2 = relu(2 - 2 cos); both sides pre-normalized so dot == cos
            d2 = small.tile([128, KNBR], FP, tag="d2")
            nc.scalar.activation(d2[:], dots[:], ACT.Relu, bias=twos[:, :1],
                                 scale=-2.0)

            # e = 2*eqv - 1, masked
            e5 = small.tile([128, KNBR], FP, tag="e5")
            nc.vector.tensor_scalar(out=e5[:], in0=eqv[:], scalar1=2.0,
                                    scalar2=-1.0, op0=ALU.mult, op1=ALU.add)
            em = small.tile([128, KNBR], FP, tag="em")
            nc.vector.tensor_tensor(out=em[:], in0=e5[:], in1=msk[:], op=ALU.mult)
            nc.vector.tensor_tensor(out=acc[:, KNBR * m:KNBR * (m + 1)],
                                    in0=em[:], in1=d2[:], op=ALU.mult)

        # ---- hpT load with m0 matmuls riding it ----
        sims[0] = simp.tile([128, B_PREV], F16, tag="sim", name="sim0")
        for pair in range(N_CHUNKS // 2):
            nc.sync.dma_start(hpT[:, 2 * pair], hpT_d[2 * pair].rearrange(
                "p (k i j) -> p k i j", k=K3, i=2))
            nc.sync.dma_start(hpT[:, 2 * pair + 1], hpT_d[2 * pair + 1].rearrange(
                "p (k i j) -> p k i j", k=K3, i=2))
            mm_pair(0, pair)

        topk_gather(0)
        for m in (1, 2, 3):
            sims[m] = simp.tile([128, B_PREV], F16, tag="sim", name=f"sim{m}")
            for pair in range(N_CHUNKS // 2):
                mm_pair(m, pair)
            topk_gather(m)

        # ---- final reduction: acc [128, 20] -> scalar ----
        rowsum = small.tile([128, 1], FP, tag="rowsum")
        nc.vector.tensor_reduce(out=rowsum[:], in_=acc[:],
                                axis=mybir.AxisListType.X, op=ALU.add)
        pps = psp1.tile([1, 1], FP, tag="pps")
        nc.tensor.matmul(pps[:], ones[:], rowsum[:], start=True, stop=True)
        res = small.tile([1, 1], FP, tag="res")
        nc.scalar.copy(res[:], pps[:])
        sc = small.tile([1, 1], FP, tag="sc")
        nc.vector.tensor_scalar_mul(sc[:], res[:], WEIGHT / (B_CURR * B_CURR))
        nc.sync.dma_start(partial[:], sc[:])

        for p in (psp1, psp, gathp, small, simp, sb):
            p.release()

    nc.compile()
    return nc


def _get_nc():
    if "nc" not in _CACHE:
        _CACHE["nc"] = _build()
    return _CACHE["nc"]


def _in_maps(hidden_current, hidden_previous, labels_previous):
    hp = np.asarray(hidden_previous, dtype=np.float32)
    hc = np.asarray(hidden_current, dtype=np.float32)
    lp = np.asarray(labels_previous).astype(np.float32)

    hpn = hp / np.maximum(np.linalg.norm(hp, axis=1, keepdims=True), 1e-12)
    hcn = hc / np.maximum(np.linalg.norm(hc, axis=1, keepdims=True), 1e-12)
    hp8 = (hpn * F8_SCALE).astype(F8_NP)
    hcn_bf = hcn.astype(BF_NP)

    # [n, p, k3, i, j2] = hp8[512n+j2, 256k3+128i+p]
    hpT_host = np.ascontiguousarray(
        hp8.reshape(N_CHUNKS, NC_CHUNK, K3, 2, 128).transpose(0, 4, 2, 3, 1)
    ).reshape(N_CHUNKS, 128, K3 * 2 * NC_CHUNK)

    tbl = np.zeros((B_CURR, TBL_W), dtype=BF_NP)
    tbl[:, :D] = hcn_bf
    tbl[:, D] = lp[:B_CURR].astype(BF_NP)

    in_maps = []
    for c in range(N_CORES):
        r0 = c * ROWS_PER_CORE
        lhsT_host = np.ascontiguousarray(
            hp8[r0:r0 + ROWS_PER_CORE].reshape(ROWS_PER_CORE, K3, 2, 128)
            .transpose(3, 1, 2, 0)
        ).reshape(128, K3 * 2 * ROWS_PER_CORE)
        s_host = np.ascontiguousarray(
            hcn_bf[r0:r0 + ROWS_PER_CORE].reshape(M_TILES, 128, D)
            .transpose(1, 0, 2)
        ).reshape(128, M_TILES * D)
        lbl_host = np.ascontiguousarray(
            lp[r0:r0 + ROWS_PER_CORE].reshape(M_TILES, 128).T)
        in_maps.append({
            "hpT": hpT_host,
            "lhsT": lhsT_host,
            "s": s_host,
            "tbl": tbl,
            "lbl": lbl_host,
        })
    return in_maps


def _run(hidden_current, hidden_previous, labels_previous, trace=False):
    nc = _get_nc()
    in_maps = _in_maps(hidden_current, hidden_previous, labels_previous)
    out = run_bass_kernel_spmd(nc, in_maps, list(range(N_CORES)), trace=trace)
    total = np.float32(0.0)
    for c in range(N_CORES):
        total += out.results[c]["partial"][0, 0]
    return np.asarray(total, dtype=np.float32), out


def kernel(hidden_current, hidden_previous, labels_current, labels_previous):
    result, _ = _run(hidden_current, hidden_previous, labels_previous)
    return result
